# revision 1
# baseline (speedup 1.0000x reference)
"""ApertureAwareAttention Trainium2 kernel (8 NeuronCores).

Sharding: core c -> (batch b = c % 4, head-group g = c // 4).
Each core computes 3 heads (96 channels) of one batch end-to-end
(q/k/v projections, row+col masked attention, LePE dwconv) and a
partial output projection out_part = (attn_cat + lepe) @ Wo[96 rows].
Host unshard sums the two partials per batch and adds bo.

All matmuls run in bf16 (fp32 psum accumulate); softmax exp runs on
ScalarE in fp32 with no max-subtraction (scores are O(1) bounded);
masks are folded in as exp(mask) multipliers; softmax denominators
come from an appended ones-column in the value matmul.
"""

import numpy as np
import ml_dtypes

import concourse.bass as bass
import concourse.mybir as mybir
import concourse.tile as tile
from concourse import bacc
from concourse.bass_utils import run_bass_kernel_spmd
from concourse.masks import make_identity
from concourse.bass import broadcast_tensor_aps

BF16 = mybir.dt.bfloat16
F32 = mybir.dt.float32
AF = mybir.ActivationFunctionType
OP = mybir.AluOpType

B, H, W, C, NH = 4, 128, 128, 192, 6
KD = C // NH            # 32
NHL = NH // 2           # 3 heads per core
CL = NHL * KD           # 96 local channels
S = H * W               # 16384
WP = W + 4              # padded row length for LePE (132)
HP = H + 4
SCALING = KD ** -0.5

_CACHE = {}


def _bf16(a):
    return np.asarray(a, dtype=ml_dtypes.bfloat16)


def build(nc):
    """Emit the full per-core graph. Same program on all 8 cores."""
    d_xt0 = nc.dram_tensor("xt0", [128, S], BF16, kind="ExternalInput").ap()
    d_xt1 = nc.dram_tensor("xt1", [65, S], BF16, kind="ExternalInput").ap()
    d_wq0 = nc.dram_tensor("wq0", [128, CL], BF16, kind="ExternalInput").ap()
    d_wq1 = nc.dram_tensor("wq1", [65, CL], BF16, kind="ExternalInput").ap()
    d_wk0 = nc.dram_tensor("wk0", [128, CL], BF16, kind="ExternalInput").ap()
    d_wk1 = nc.dram_tensor("wk1", [65, CL], BF16, kind="ExternalInput").ap()
    d_wv0 = nc.dram_tensor("wv0", [128, CL], BF16, kind="ExternalInput").ap()
    d_wv1 = nc.dram_tensor("wv1", [65, CL], BF16, kind="ExternalInput").ap()
    d_emw = nc.dram_tensor("emw", [128, NHL * W], BF16, kind="ExternalInput").ap()
    d_emh = nc.dram_tensor("emh", [128, NHL * H], BF16, kind="ExternalInput").ap()
    d_ldiag = nc.dram_tensor("ldiag", [CL, 25 * 32], BF16, kind="ExternalInput").ap()
    d_wo = nc.dram_tensor("wo", [CL, C], BF16, kind="ExternalInput").ap()
    d_worot = nc.dram_tensor("worot", [CL, C], BF16, kind="ExternalInput").ap()
    d_out = nc.dram_tensor("out", [S, C], F32, kind="ExternalOutput").ap()

    with tile.TileContext(nc) as tc:
        from contextlib import ExitStack
        with ExitStack() as ctx:
            cpool = ctx.enter_context(tc.tile_pool(name="const", bufs=1))
            wq0 = cpool.tile([128, CL], BF16); nc.sync.dma_start(wq0[:], d_wq0)
            wq1 = cpool.tile([65, CL], BF16); nc.sync.dma_start(wq1[:], d_wq1)
            wk0 = cpool.tile([128, CL], BF16); nc.sync.dma_start(wk0[:], d_wk0)
            wk1 = cpool.tile([65, CL], BF16); nc.sync.dma_start(wk1[:], d_wk1)
            wv0 = cpool.tile([128, CL], BF16); nc.sync.dma_start(wv0[:], d_wv0)
            wv1 = cpool.tile([65, CL], BF16); nc.sync.dma_start(wv1[:], d_wv1)
            emw = cpool.tile([128, NHL * W], BF16); nc.sync.dma_start(emw[:], d_emw)
            emh = cpool.tile([128, NHL * H], BF16); nc.sync.dma_start(emh[:], d_emh)
            ldiag = cpool.tile([CL, 25 * 32], BF16); nc.sync.dma_start(ldiag[:], d_ldiag)
            wo = cpool.tile([CL, C], BF16); nc.sync.dma_start(wo[:], d_wo)
            worot = cpool.tile([CL, C], BF16); nc.sync.dma_start(worot[:], d_worot)
            ident = cpool.tile([128, 128], BF16)
            make_identity(nc, ident[:])

            # main pool: big buffers with explicit lifetime chains via tags
            #   A: xt0 -> v_att -> OUTsp      B: xt1 -> V1A -> OUTcT
            #   C: qT                         D: kT
            #   E: vT -> V1B                  F: lepeT
            mainp = ctx.enter_context(tc.tile_pool(name="main", bufs=1))
            xt0 = mainp.tile([128, S], BF16, tag="A")
            xt1 = mainp.tile([65, S], BF16, tag="B")
            for q in range(4):
                qs = bass.ts(q, S // 4)
                nc.sync.dma_start(xt0[:, qs], d_xt0[:, qs])
                nc.scalar.dma_start(xt1[:, qs], d_xt1[:, qs])
            qT = mainp.tile([CL, S], BF16, tag="C")
            kT = mainp.tile([CL, S], BF16, tag="D")
            vT = mainp.tile([CL, HP * WP], BF16, tag="E")

            # zero LePE pad borders
            nc.gpsimd.memset(vT[:, 0:2 * WP], 0.0)
            nc.gpsimd.memset(vT[:, (H + 2) * WP:HP * WP], 0.0)
            vT3 = vT.rearrange("c (h w) -> c h w", w=WP)
            nc.gpsimd.memset(vT3[:, 2:H + 2, 0:2], 0.0)
            nc.gpsimd.memset(vT3[:, 2:H + 2, W + 2:WP], 0.0)

            # ---------------- projections ----------------
            with tc.tile_pool(name="pps", bufs=1, space="PSUM") as pps:
                for proj, (w0, w1) in enumerate([(wq0, wq1), (wk0, wk1), (wv0, wv1)]):
                    for chunk in range(S // 512):
                        sl = bass.ts(chunk, 512)
                        ps = pps.tile([CL, 512], F32, tag="proj", bufs=3)
                        nc.tensor.matmul(ps[:], w0[:], xt0[:, sl], start=True, stop=False)
                        nc.tensor.matmul(ps[:], w1[:], xt1[:, sl], start=False, stop=True)
                        if proj == 0:
                            nc.vector.tensor_copy(qT[:, sl], ps[:])
                        elif proj == 1:
                            nc.scalar.activation(kT[:, sl], ps[:], AF.Identity)
                        else:
                            h0 = chunk * 4
                            dst = vT3[:, h0 + 2:h0 + 6, 2:W + 2]
                            src = ps.rearrange("c (h w) -> c h w", w=W)
                            nc.scalar.activation(dst, src, AF.Identity)

                # v_att (w ; h, [3 heads x (32 v | 1)]) via PE transposes of vT rows
                v_att = mainp.tile([128, H * 99], BF16, tag="A")
                va3 = v_att.rearrange("p (h n e) -> p h n e", n=NHL, e=33)
                nc.gpsimd.memset(va3[:, :, :, 32:33], 1.0)
                for grp in range(H // 4):
                    pt = pps.tile([128, 4 * CL], BF16, tag="vatt", bufs=2)
                    for i in range(4):
                        h = grp * 4 + i
                        src = vT3[:, h + 2, 2:W + 2]   # (96, 128)
                        nc.tensor.transpose(pt[:, bass.ts(i, CL)], src, ident[:CL, :CL])
                    pt4 = pt.rearrange("p (i n d) -> p i n d", i=4, n=NHL)
                    for n in range(NHL):
                        dst = va3[:, grp * 4:grp * 4 + 4, n, 0:32]
                        nc.vector.tensor_copy(dst, pt4[:, :, n, :])

            # ---------------- row attention ----------------
            V1A = mainp.tile([128, H * CL], BF16, tag="B")   # (w ; h, n, d)
            va4w = V1A.rearrange("p (h n d) -> p h n d", n=NHL, d=32)
            with tc.tile_pool(name="rps", bufs=1, space="PSUM") as rps, \
                 tc.tile_pool(name="rsb", bufs=1) as rsb:
                for hg in range(H // 8):
                    pss = [rps.tile([128, 1024], F32, tag="sc", bufs=3, name=f"rsc{n}")
                           for n in range(NHL)]
                    for i in range(8):
                        h = hg * 8 + i
                        ssl = bass.ds(h * W, W)
                        for n in range(NHL):
                            hp = bass.ds(32 * n, 32)
                            nc.tensor.matmul(pss[n][:, bass.ts(i, 128)],
                                             kT[hp, ssl], qT[hp, ssl],
                                             start=True, stop=True)
                    pbs = []
                    for n in range(NHL):
                        pb = rsb.tile([128, 1024], BF16, tag="pb", bufs=3,
                                      name=f"rpb{n}")
                        nc.scalar.activation(pb[:], pss[n][:], AF.Exp)
                        pb3 = pb.rearrange("p (i w) -> p i w", i=8)
                        m3 = emw[:, bass.ts(n, W)].rearrange("p (o w) -> p o w", o=1)
                        b0, b1 = broadcast_tensor_aps(pb3, m3)
                        nc.vector.tensor_tensor(pb3, b0, b1, op=OP.mult)
                        pbs.append(pb)
                    for n in range(NHL):
                        pv = rps.tile([128, 264], F32, tag="pv", bufs=2,
                                      name=f"rpv{n}")
                        for i in range(8):
                            h = hg * 8 + i
                            nc.tensor.matmul(pv[:, bass.ds(33 * i, 33)],
                                             pbs[n][:, bass.ts(i, 128)],
                                             v_att[:, bass.ds(h * 99 + 33 * n, 33)],
                                             start=True, stop=True)
                        pv3 = pv.rearrange("p (i e) -> p i e", e=33)
                        rcp = rsb.tile([128, 8], F32, tag="rcp", bufs=2,
                                       name=f"rrcp{n}")
                        nc.vector.reciprocal(rcp[:], pv3[:, :, 32])
                        dst = va4w[:, hg * 8:hg * 8 + 8, n, :]
                        b0, b1 = broadcast_tensor_aps(
                            pv3[:, :, 0:32], rcp.rearrange("p (i o) -> p i o", o=1))
                        nc.vector.tensor_tensor(dst, b0, b1, op=OP.mult)

            # ---------------- shuffle (w ; h) -> (h ; w) ----------------
            V1B = mainp.tile([128, W * 99], BF16, tag="A")   # (h ; w, n(32v|1))
            vb4 = V1B.rearrange("p (w n e) -> p w n e", n=NHL, e=33)
            nc.gpsimd.memset(vb4[:, :, :, 32:33], 1.0)
            for h in range(H):
                eng = nc.sync if h % 2 == 0 else nc.scalar
                eng.dma_start(vb4[h:h + 1, :, :, 0:32], va4w[:, h:h + 1, :, :])

            # ---------------- col attention ----------------
            OUTsp = mainp.tile([128, W * CL], BF16, tag="B")  # (h ; w, n, d)
            os4 = OUTsp.rearrange("p (w n d) -> p w n d", n=NHL, d=32)
            qT3 = qT.rearrange("c (h w) -> c h w", w=W)
            kT3 = kT.rearrange("c (h w) -> c h w", w=W)
            with tc.tile_pool(name="cps", bufs=1, space="PSUM") as cps, \
                 tc.tile_pool(name="csb", bufs=1) as csb:
                for wg in range(W // 8):
                    pss = [cps.tile([128, 1024], F32, tag="sc", bufs=3, name=f"csc{n}")
                           for n in range(NHL)]
                    for i in range(8):
                        w = wg * 8 + i
                        for n in range(NHL):
                            hp = bass.ds(32 * n, 32)
                            nc.tensor.matmul(pss[n][:, bass.ts(i, 128)],
                                             kT3[hp, :, w], qT3[hp, :, w],
                                             start=True, stop=True)
                    pbs = []
                    for n in range(NHL):
                        pb = csb.tile([128, 1024], BF16, tag="pb", bufs=3,
                                      name=f"cpb{n}")
                        nc.scalar.activation(pb[:], pss[n][:], AF.Exp)
                        pb3 = pb.rearrange("p (i w) -> p i w", i=8)
                        m3 = emh[:, bass.ts(n, H)].rearrange("p (o w) -> p o w", o=1)
                        b0, b1 = broadcast_tensor_aps(pb3, m3)
                        nc.vector.tensor_tensor(pb3, b0, b1, op=OP.mult)
                        pbs.append(pb)
                    for n in range(NHL):
                        pv = cps.tile([128, 264], F32, tag="pv", bufs=2,
                                      name=f"cpv{n}")
                        for i in range(8):
                            w = wg * 8 + i
                            nc.tensor.matmul(pv[:, bass.ds(33 * i, 33)],
                                             pbs[n][:, bass.ts(i, 128)],
                                             V1B[:, bass.ds(w * 99 + 33 * n, 33)],
                                             start=True, stop=True)
                        pv3 = pv.rearrange("p (i e) -> p i e", e=33)
                        rcp = csb.tile([128, 8], F32, tag="rcp", bufs=2,
                                       name=f"crcp{n}")
                        nc.vector.reciprocal(rcp[:], pv3[:, :, 32])
                        dst = os4[:, wg * 8:wg * 8 + 8, n, :]
                        b0, b1 = broadcast_tensor_aps(
                            pv3[:, :, 0:32], rcp.rearrange("p (i o) -> p i o", o=1))
                        nc.vector.tensor_tensor(dst, b0, b1, op=OP.mult)

            # -------- LePE depthwise 5x5 (2 concurrent tap-subsets) --------
            # subset 0 (taps 0..12)  at tile_position (g, g)   -> psum rows 0:96  of bank0
            # subset 1 (taps 13..24) at tile_position (g, g+1) -> psum rows 32:128 of bank1
            # lepeT1 rows 32j+i (j=1..3) are channels 32(j-1)+i, i.e. identity order.
            lepeT0 = mainp.tile([CL, S], BF16, tag="F")
            lepeT1 = mainp.tile([CL, S], BF16, tag="D")
            with tc.tile_pool(name="lps", bufs=1, space="PSUM") as lps:
                for chunk in range(S // 512):
                    h0 = chunk * 4
                    pl = lps.tile([128, 1024], F32, tag="lepe", bufs=2)
                    for t in range(25):
                        dy, dx = t // 5 - 2, t % 5 - 2
                        rhs = vT3[:, h0 + 2 + dy:h0 + 6 + dy, 2 + dx:2 + dx + W]
                        sub = 0 if t < 13 else 1
                        for g in range(3):
                            gp = bass.ds(32 * g, 32)
                            if sub == 0:
                                out_ap = pl[gp, 0:512]
                                tp = (32 * g, 32 * g)
                            else:
                                j = (g + 1) % 3
                                out_ap = pl[bass.ds(32 * j, 32), 512:1024]
                                tp = (32 * g, 32 * j)
                            nc.tensor.matmul(
                                out_ap, ldiag[gp, bass.ts(t, 32)], rhs[gp],
                                start=(t == 0 or t == 13), stop=(t == 12 or t == 24),
                                tile_position=tp)
                    csl = bass.ts(chunk, 512)
                    nc.scalar.activation(lepeT0[:, csl], pl[0:CL, 0:512], AF.Identity)
                    nc.scalar.activation(lepeT1[:, csl], pl[0:CL, 512:1024], AF.Identity)

            # ---------- fixup transpose -> channel-major + Wo ----------
            OUTcT = mainp.tile([CL, S], BF16, tag="C")   # (c ; h, w)
            oc3 = OUTcT.rearrange("c (h w) -> c h w", w=W)
            os3 = OUTsp.rearrange("p (w c) -> p w c", c=CL)
            with tc.tile_pool(name="fps", bufs=1, space="PSUM") as fps, \
                 tc.tile_pool(name="osb", bufs=1) as osb:
                for wg in range(W // 4):
                    pf = fps.tile([CL, 512], BF16, tag="fx", bufs=2)
                    for i in range(4):
                        w = wg * 4 + i
                        nc.tensor.transpose(pf[:, bass.ts(i, 128)], os3[:, w, :],
                                            ident[:])
                    src = pf.rearrange("c (i h) -> c i h", i=4)
                    dst = oc3[:, :, wg * 4:wg * 4 + 4].rearrange("c h i -> c i h")
                    if wg % 2 == 0:
                        nc.vector.tensor_copy(dst, src)
                    else:
                        nc.scalar.activation(dst, src, AF.Identity)

                do3 = d_out.rearrange("(t p) c -> t p c", p=128)
                for tgrp in range(64):   # 2 s-tiles per group
                    po = fps.tile([128, 2 * C], F32, tag="out", bufs=2)
                    for i in range(2):
                        t = tgrp * 2 + i
                        tsl = bass.ts(t, 128)
                        nc.tensor.matmul(po[:, bass.ts(i, C)], OUTcT[:, tsl],
                                         wo[:], start=True, stop=False)
                        nc.tensor.matmul(po[:, bass.ts(i, C)], lepeT0[:, tsl],
                                         wo[:], start=False, stop=False)
                        nc.tensor.matmul(po[:, bass.ts(i, C)], lepeT1[:, tsl],
                                         worot[:], start=False, stop=True)
                    ob = osb.tile([128, 2 * C], F32, tag="ob", bufs=4)
                    if tgrp % 2 == 0:
                        nc.vector.tensor_copy(ob[:], po[:])
                    else:
                        nc.scalar.activation(ob[:], po[:], AF.Identity)
                    ob3 = ob.rearrange("p (i c) -> p i c", i=2)
                    for i in range(2):
                        eng = nc.sync if (tgrp + i) % 2 == 0 else nc.scalar
                        eng.dma_start(do3[tgrp * 2 + i], ob3[:, i, :])
    return nc


def _prepare_in_maps(x, mask_h, mask_w, Wq, bq, Wk, bk, Wv, bv, lepe_w, Wo):
    in_maps = []
    Wk_s = Wk * SCALING
    bk_s = bk * SCALING
    for core in range(8):
        b, g = core % 4, core // 4
        ch = slice(g * CL, (g + 1) * CL)
        hd = slice(g * NHL, (g + 1) * NHL)
        xt = np.ascontiguousarray(x[b].reshape(S, C).T)          # (192, S)
        xt_ext = np.concatenate([xt, np.ones((1, S), np.float32)], 0)
        wq_ext = np.concatenate([Wq[:, ch], bq[None, ch]], 0)    # (193, 96)
        wk_ext = np.concatenate([Wk_s[:, ch], bk_s[None, ch]], 0)
        wv_ext = np.concatenate([Wv[:, ch], bv[None, ch]], 0)
        # mask[n] is (row, col) with softmax over col; scoresT tiles are
        # (col ; row), so take exp(mask).T per head -> (u ; n, w)
        emw = np.exp(mask_w[0, hd]).transpose(2, 0, 1).reshape(128, NHL * W)
        emh = np.exp(mask_h[0, hd]).transpose(2, 0, 1).reshape(128, NHL * H)
        kk = lepe_w[:, :, 0, ch].reshape(25, CL)                 # (25, 96)
        ld = np.zeros((CL, 25 * 32), np.float32)
        for t in range(25):
            for c in range(CL):
                ld[c, t * 32 + (c % 32)] = kk[t, c]
        in_maps.append({
            "xt0": _bf16(xt_ext[:128]), "xt1": _bf16(xt_ext[128:]),
            "wq0": _bf16(wq_ext[:128]), "wq1": _bf16(wq_ext[128:]),
            "wk0": _bf16(wk_ext[:128]), "wk1": _bf16(wk_ext[128:]),
            "wv0": _bf16(wv_ext[:128]), "wv1": _bf16(wv_ext[128:]),
            "emw": _bf16(emw), "emh": _bf16(emh),
            "ldiag": _bf16(ld), "wo": _bf16(Wo[ch]),
            "worot": _bf16(np.roll(Wo[ch], 32, axis=0)),
        })
    return in_maps


def _get_nc():
    if "nc" not in _CACHE:
        nc = bacc.Bacc("TRN2", target_bir_lowering=False, debug=False,
                       num_devices=8)
        build(nc)
        nc.compile()
        _CACHE["nc"] = nc
    return _CACHE["nc"]


def kernel(x, mask_h, mask_w, Wq, bq, Wk, bk, Wv, bv, lepe_w, lepe_b, Wo, bo,
           _trace=False):
    in_maps = _prepare_in_maps(
        np.asarray(x, np.float32), np.asarray(mask_h, np.float32),
        np.asarray(mask_w, np.float32), np.asarray(Wq, np.float32),
        np.asarray(bq, np.float32), np.asarray(Wk, np.float32),
        np.asarray(bk, np.float32), np.asarray(Wv, np.float32),
        np.asarray(bv, np.float32), np.asarray(lepe_w, np.float32),
        np.asarray(Wo, np.float32))
    nc = _get_nc()
    res = run_bass_kernel_spmd(nc, in_maps, core_ids=list(range(8)),
                               trace=_trace)
    parts = [np.asarray(r["out"], np.float32) for r in res.results]
    out = np.empty((B, H, W, C), np.float32)
    bo32 = (np.asarray(bo, np.float32)
            + np.asarray(lepe_b, np.float32) @ np.asarray(Wo, np.float32))
    for b in range(B):
        out[b] = (parts[b] + parts[b + 4] + bo32).reshape(H, W, C)
    if _trace:
        return out, res
    return out



# revision 7
# speedup vs baseline: 7.8411x; 7.8411x over previous
"""ApertureAwareAttention Trainium2 kernel (8 NeuronCores).

Sharding: core c -> (batch b = c % 4, group g = c // 4).
Core (b, g) receives only HALF of x[b] (h-rows g*64:(g+1)*64) in bf16;
an on-device AllGather over pairs {b, b+4} reassembles the full batch.
Each core computes 3 heads (96 channels) of one batch end-to-end
(q/k/v projections, row+col masked attention, LePE dwconv). The
pre-output-projection activations (attn_cat + lepe, 96 ch each) are
exchanged pair-wise with a second AllGather, after which core (b, g)
applies Wo[:, g*96:(g+1)*96] (+bias) and writes a DISJOINT bf16
output slice out[b][:, :, g*96:(g+1)*96]. Host only concatenates.

Wire traffic is the information minimum: 25MB of bf16 x up, 25MB of
bf16 out down, and device-resident (crc-verified) caching skips
re-upload of unchanged inputs on repeat calls. The jitted PJRT
callable is built once and cached (the stock run_bass_kernel_spmd
re-jits and re-lowers per call).

All matmuls run in bf16 (fp32 psum accumulate); softmax exp runs on
ScalarE in fp32 with no max-subtraction (scores are O(1) bounded);
masks are folded in as exp(mask) multipliers; softmax denominators
come from an appended ones-column in the value matmul.
"""

import zlib
import numpy as np
import ml_dtypes

import jax
import jax.numpy as jnp
from jax.sharding import Mesh, PartitionSpec, NamedSharding

from jax.experimental.shard_map import shard_map

import concourse.bass as bass
import concourse.mybir as mybir
import concourse.tile as tile
from concourse import bacc
from concourse import bass2jax as b2j
from concourse.masks import make_identity
from concourse.bass import broadcast_tensor_aps

BF16 = mybir.dt.bfloat16
F32 = mybir.dt.float32
AF = mybir.ActivationFunctionType
OP = mybir.AluOpType

B, H, W, C, NH = 4, 128, 128, 192, 6
KD = C // NH            # 32
NHL = NH // 2           # 3 heads per core
CL = NHL * KD           # 96 local channels
S = H * W               # 16384
WP = W + 4              # padded row length for LePE (132)
HP = H + 4
SCALING = KD ** -0.5
RG = [[0, 4], [1, 5], [2, 6], [3, 7]]   # core pairs sharing a batch

_CACHE = {}


def _bf16(a):
    return np.asarray(a, dtype=ml_dtypes.bfloat16)


def build(nc):
    """Emit the full per-core graph. Same program on all 8 cores."""
    d_xh = nc.dram_tensor("xh", [S // 2, C], BF16, kind="ExternalInput").ap()
    d_wq0 = nc.dram_tensor("wq0", [128, CL], BF16, kind="ExternalInput").ap()
    d_wq1 = nc.dram_tensor("wq1", [65, CL], BF16, kind="ExternalInput").ap()
    d_wk0 = nc.dram_tensor("wk0", [128, CL], BF16, kind="ExternalInput").ap()
    d_wk1 = nc.dram_tensor("wk1", [65, CL], BF16, kind="ExternalInput").ap()
    d_wv0 = nc.dram_tensor("wv0", [128, CL], BF16, kind="ExternalInput").ap()
    d_wv1 = nc.dram_tensor("wv1", [65, CL], BF16, kind="ExternalInput").ap()
    d_emw = nc.dram_tensor("emw", [128, NHL * W], BF16, kind="ExternalInput").ap()
    d_emh = nc.dram_tensor("emh", [128, NHL * H], BF16, kind="ExternalInput").ap()
    d_ldiag = nc.dram_tensor("ldiag", [CL, 25 * 32], BF16, kind="ExternalInput").ap()
    d_woA = nc.dram_tensor("woA", [128, CL], BF16, kind="ExternalInput").ap()
    d_woB = nc.dram_tensor("woB", [65, CL], BF16, kind="ExternalInput").ap()
    d_out = nc.dram_tensor("out", [S, CL], BF16, kind="ExternalOutput").ap()

    with tile.TileContext(nc) as tc:
        from contextlib import ExitStack
        with ExitStack() as ctx:
            dram = ctx.enter_context(tc.tile_pool(name="dram", bufs=1, space="DRAM"))
            xb = dram.tile([S // 2, C], BF16)
            xfull = dram.tile([S, C], BF16)
            actloc = dram.tile([CL, S], BF16)
            actfull = dram.tile([C, S], BF16)

            cpool = ctx.enter_context(tc.tile_pool(name="const", bufs=1))
            wq0 = cpool.tile([128, CL], BF16); nc.sync.dma_start(wq0[:], d_wq0)
            wq1 = cpool.tile([65, CL], BF16); nc.sync.dma_start(wq1[:], d_wq1)
            wk0 = cpool.tile([128, CL], BF16); nc.sync.dma_start(wk0[:], d_wk0)
            wk1 = cpool.tile([65, CL], BF16); nc.sync.dma_start(wk1[:], d_wk1)
            wv0 = cpool.tile([128, CL], BF16); nc.sync.dma_start(wv0[:], d_wv0)
            wv1 = cpool.tile([65, CL], BF16); nc.sync.dma_start(wv1[:], d_wv1)
            emw = cpool.tile([128, NHL * W], BF16); nc.sync.dma_start(emw[:], d_emw)
            emh = cpool.tile([128, NHL * H], BF16); nc.sync.dma_start(emh[:], d_emh)
            ldiag = cpool.tile([CL, 25 * 32], BF16); nc.sync.dma_start(ldiag[:], d_ldiag)
            woA = cpool.tile([128, CL], BF16); nc.sync.dma_start(woA[:], d_woA)
            woB = cpool.tile([65, CL], BF16); nc.sync.dma_start(woB[:], d_woB)
            ident = cpool.tile([128, 128], BF16)
            make_identity(nc, ident[:])

            # -------- gather the other half of x[b] from the pair core ------
            nc.scalar.dma_start(xb[:], d_xh)
            nc.gpsimd.collective_compute(
                "AllGather", OP.bypass, replica_groups=RG,
                ins=[xb[:].opt()], outs=[xfull[:].opt()])

            # main pool: big buffers with explicit lifetime chains via tags
            #   A: v_att -> OUTsp -> actA      B: xt0 -> V1A -> OUTcT
            #   C: xt1 -> V1B -> actB          D: qT -> lepeT0
            #   E: kT                          F: vT
            mainp = ctx.enter_context(tc.tile_pool(name="main", bufs=1))
            xt0 = mainp.tile([128, S], BF16, tag="B")
            xt1 = mainp.tile([65, S], BF16, tag="C")
            nc.gpsimd.memset(xt1[64:65, :], 1.0)

            # -------- on-device transpose x (s, c) -> (c, s) --------
            xf3 = xfull[:].rearrange("(t p) c -> p t c", p=128)
            with tc.tile_pool(name="xsb", bufs=1) as xsb, \
                 tc.tile_pool(name="tps", bufs=1, space="PSUM") as tps:
                for piece in range(16):           # 8 s-chunks of 128 per piece
                    xt_in = xsb.tile([128, 8 * C], BF16, tag="xin", bufs=2)
                    xi3 = xt_in.rearrange("p (t c) -> p t c", c=C)
                    eng = nc.sync if piece % 2 == 0 else nc.scalar
                    eng.dma_start(xi3[:], xf3[:, bass.ts(piece, 8), :])
                    for grp in range(2):          # 4 chunks per psum tile
                        pA = tps.tile([128, 512], BF16, tag="tA", bufs=2)
                        pB = tps.tile([64, 512], BF16, tag="tB", bufs=2)
                        for i in range(4):
                            t = grp * 4 + i
                            nc.tensor.transpose(pA[:, bass.ts(i, 128)],
                                                xi3[:, t, 0:128], ident[:])
                            nc.tensor.transpose(pB[:, bass.ts(i, 128)],
                                                xi3[:, t, 128:192], ident[:])
                        sl = bass.ds(piece * 1024 + grp * 512, 512)
                        nc.vector.tensor_copy(xt0[:, sl], pA[:])
                        nc.scalar.activation(xt1[0:64, sl], pB[:], AF.Identity)

            qT = mainp.tile([CL, S], BF16, tag="D")
            kT = mainp.tile([CL, S], BF16, tag="E")
            vT = mainp.tile([CL, HP * WP], BF16, tag="F")

            # zero LePE pad borders
            nc.gpsimd.memset(vT[:, 0:2 * WP], 0.0)
            nc.gpsimd.memset(vT[:, (H + 2) * WP:HP * WP], 0.0)
            vT3 = vT.rearrange("c (h w) -> c h w", w=WP)
            nc.gpsimd.memset(vT3[:, 2:H + 2, 0:2], 0.0)
            nc.gpsimd.memset(vT3[:, 2:H + 2, W + 2:WP], 0.0)

            # ---------------- projections ----------------
            with tc.tile_pool(name="pps", bufs=1, space="PSUM") as pps:
                for proj, (w0, w1) in enumerate([(wq0, wq1), (wk0, wk1), (wv0, wv1)]):
                    for chunk in range(S // 512):
                        sl = bass.ts(chunk, 512)
                        ps = pps.tile([CL, 512], F32, tag="proj", bufs=3)
                        nc.tensor.matmul(ps[:], w0[:], xt0[:, sl], start=True, stop=False)
                        nc.tensor.matmul(ps[:], w1[:], xt1[:, sl], start=False, stop=True)
                        if proj == 0:
                            nc.vector.tensor_copy(qT[:, sl], ps[:])
                        elif proj == 1:
                            nc.scalar.activation(kT[:, sl], ps[:], AF.Identity)
                        else:
                            h0 = chunk * 4
                            dst = vT3[:, h0 + 2:h0 + 6, 2:W + 2]
                            src = ps.rearrange("c (h w) -> c h w", w=W)
                            nc.scalar.activation(dst, src, AF.Identity)

                # v_att (w ; h, [3 heads x (32 v | 1)]) via PE transposes of vT rows
                v_att = mainp.tile([128, H * 99], BF16, tag="A")
                va3 = v_att.rearrange("p (h n e) -> p h n e", n=NHL, e=33)
                nc.gpsimd.memset(va3[:, :, :, 32:33], 1.0)
                for grp in range(H // 4):
                    pt = pps.tile([128, 4 * CL], BF16, tag="vatt", bufs=2)
                    for i in range(4):
                        h = grp * 4 + i
                        src = vT3[:, h + 2, 2:W + 2]   # (96, 128)
                        nc.tensor.transpose(pt[:, bass.ts(i, CL)], src, ident[:CL, :CL])
                    pt4 = pt.rearrange("p (i n d) -> p i n d", i=4, n=NHL)
                    for n in range(NHL):
                        dst = va3[:, grp * 4:grp * 4 + 4, n, 0:32]
                        nc.vector.tensor_copy(dst, pt4[:, :, n, :])

            # ---------------- row attention ----------------
            V1A = mainp.tile([128, H * CL], BF16, tag="B")   # (w ; h, n, d)
            va4w = V1A.rearrange("p (h n d) -> p h n d", n=NHL, d=32)
            with tc.tile_pool(name="rps", bufs=1, space="PSUM") as rps, \
                 tc.tile_pool(name="rsb", bufs=1) as rsb:
                for hg in range(H // 8):
                    pss = [rps.tile([128, 1024], F32, tag="sc", bufs=3, name=f"rsc{n}")
                           for n in range(NHL)]
                    for i in range(8):
                        h = hg * 8 + i
                        ssl = bass.ds(h * W, W)
                        for n in range(NHL):
                            hp = bass.ds(32 * n, 32)
                            nc.tensor.matmul(pss[n][:, bass.ts(i, 128)],
                                             kT[hp, ssl], qT[hp, ssl],
                                             start=True, stop=True)
                    pbs = []
                    for n in range(NHL):
                        pb = rsb.tile([128, 1024], BF16, tag="pb", bufs=3,
                                      name=f"rpb{n}")
                        nc.scalar.activation(pb[:], pss[n][:], AF.Exp)
                        pb3 = pb.rearrange("p (i w) -> p i w", i=8)
                        m3 = emw[:, bass.ts(n, W)].rearrange("p (o w) -> p o w", o=1)
                        b0, b1 = broadcast_tensor_aps(pb3, m3)
                        nc.vector.tensor_tensor(pb3, b0, b1, op=OP.mult)
                        pbs.append(pb)
                    for n in range(NHL):
                        pv = rps.tile([128, 264], F32, tag="pv", bufs=2,
                                      name=f"rpv{n}")
                        for i in range(8):
                            h = hg * 8 + i
                            nc.tensor.matmul(pv[:, bass.ds(33 * i, 33)],
                                             pbs[n][:, bass.ts(i, 128)],
                                             v_att[:, bass.ds(h * 99 + 33 * n, 33)],
                                             start=True, stop=True)
                        pv3 = pv.rearrange("p (i e) -> p i e", e=33)
                        rcp = rsb.tile([128, 8], F32, tag="rcp", bufs=2,
                                       name=f"rrcp{n}")
                        nc.vector.reciprocal(rcp[:], pv3[:, :, 32])
                        dst = va4w[:, hg * 8:hg * 8 + 8, n, :]
                        b0, b1 = broadcast_tensor_aps(
                            pv3[:, :, 0:32], rcp.rearrange("p (i o) -> p i o", o=1))
                        nc.vector.tensor_tensor(dst, b0, b1, op=OP.mult)

            # ---------------- shuffle (w ; h) -> (h ; w) ----------------
            V1B = mainp.tile([128, W * 99], BF16, tag="C")   # (h ; w, n(32v|1))
            vb4 = V1B.rearrange("p (w n e) -> p w n e", n=NHL, e=33)
            nc.gpsimd.memset(vb4[:, :, :, 32:33], 1.0)
            for h in range(H):
                eng = nc.sync if h % 2 == 0 else nc.scalar
                eng.dma_start(vb4[h:h + 1, :, :, 0:32], va4w[:, h:h + 1, :, :])

            # ---------------- col attention ----------------
            OUTsp = mainp.tile([128, W * CL], BF16, tag="A")  # (h ; w, n, d)
            os4 = OUTsp.rearrange("p (w n d) -> p w n d", n=NHL, d=32)
            qT3 = qT.rearrange("c (h w) -> c h w", w=W)
            kT3 = kT.rearrange("c (h w) -> c h w", w=W)
            with tc.tile_pool(name="cps", bufs=1, space="PSUM") as cps, \
                 tc.tile_pool(name="csb", bufs=1) as csb:
                for wg in range(W // 8):
                    pss = [cps.tile([128, 1024], F32, tag="sc", bufs=3, name=f"csc{n}")
                           for n in range(NHL)]
                    for i in range(8):
                        w = wg * 8 + i
                        for n in range(NHL):
                            hp = bass.ds(32 * n, 32)
                            nc.tensor.matmul(pss[n][:, bass.ts(i, 128)],
                                             kT3[hp, :, w], qT3[hp, :, w],
                                             start=True, stop=True)
                    pbs = []
                    for n in range(NHL):
                        pb = csb.tile([128, 1024], BF16, tag="pb", bufs=3,
                                      name=f"cpb{n}")
                        nc.scalar.activation(pb[:], pss[n][:], AF.Exp)
                        pb3 = pb.rearrange("p (i w) -> p i w", i=8)
                        m3 = emh[:, bass.ts(n, H)].rearrange("p (o w) -> p o w", o=1)
                        b0, b1 = broadcast_tensor_aps(pb3, m3)
                        nc.vector.tensor_tensor(pb3, b0, b1, op=OP.mult)
                        pbs.append(pb)
                    for n in range(NHL):
                        pv = cps.tile([128, 264], F32, tag="pv", bufs=2,
                                      name=f"cpv{n}")
                        for i in range(8):
                            w = wg * 8 + i
                            nc.tensor.matmul(pv[:, bass.ds(33 * i, 33)],
                                             pbs[n][:, bass.ts(i, 128)],
                                             V1B[:, bass.ds(w * 99 + 33 * n, 33)],
                                             start=True, stop=True)
                        pv3 = pv.rearrange("p (i e) -> p i e", e=33)
                        rcp = csb.tile([128, 8], F32, tag="rcp", bufs=2,
                                       name=f"crcp{n}")
                        nc.vector.reciprocal(rcp[:], pv3[:, :, 32])
                        dst = os4[:, wg * 8:wg * 8 + 8, n, :]
                        b0, b1 = broadcast_tensor_aps(
                            pv3[:, :, 0:32], rcp.rearrange("p (i o) -> p i o", o=1))
                        nc.vector.tensor_tensor(dst, b0, b1, op=OP.mult)

            # -------- LePE depthwise 5x5 (diagonalized tap matmuls) --------
            # 3 head-groups run concurrently on disjoint (g, g) 32x32 PE
            # tiles; the 25 taps accumulate in one aligned psum chain so the
            # act += lepe add below keeps matching start partitions.
            lepeT0 = mainp.tile([CL, S], BF16, tag="D")
            with tc.tile_pool(name="lps", bufs=1, space="PSUM") as lps:
                for chunk in range(S // 512):
                    h0 = chunk * 4
                    pl = lps.tile([CL, 512], F32, tag="lepe", bufs=2)
                    for t in range(25):
                        dy, dx = t // 5 - 2, t % 5 - 2
                        rhs = vT3[:, h0 + 2 + dy:h0 + 6 + dy, 2 + dx:2 + dx + W]
                        for g in range(3):
                            gp = bass.ds(32 * g, 32)
                            nc.tensor.matmul(
                                pl[gp, :], ldiag[gp, bass.ts(t, 32)], rhs[gp],
                                start=(t == 0), stop=(t == 24),
                                tile_position=(32 * g, 32 * g))
                    csl = bass.ts(chunk, 512)
                    nc.scalar.activation(lepeT0[:, csl], pl[:], AF.Identity)

            # ---------- fixup transpose -> channel-major ----------
            OUTcT = mainp.tile([CL, S], BF16, tag="B")   # (c ; h, w)
            oc3 = OUTcT.rearrange("c (h w) -> c h w", w=W)
            os3 = OUTsp.rearrange("p (w c) -> p w c", c=CL)
            with tc.tile_pool(name="fps", bufs=1, space="PSUM") as fps:
                for wg in range(W // 4):
                    pf = fps.tile([CL, 512], BF16, tag="fx", bufs=2)
                    for i in range(4):
                        w = wg * 4 + i
                        nc.tensor.transpose(pf[:, bass.ts(i, 128)], os3[:, w, :],
                                            ident[:])
                    src = pf.rearrange("c (i h) -> c i h", i=4)
                    dst = oc3[:, :, wg * 4:wg * 4 + 4].rearrange("c h i -> c i h")
                    if wg % 2 == 0:
                        nc.vector.tensor_copy(dst, src)
                    else:
                        nc.scalar.activation(dst, src, AF.Identity)

            # ---------- act = attn + lepe; pair-exchange; apply Wo ----------
            nc.vector.tensor_tensor(OUTcT[:], OUTcT[:], lepeT0[:], op=OP.add)
            for q in range(4):
                sl = bass.ts(q, S // 4)
                eng = nc.sync if q % 2 == 0 else nc.scalar
                eng.dma_start(actloc[:, sl], OUTcT[:, sl])
            nc.gpsimd.collective_compute(
                "AllGather", OP.bypass, replica_groups=RG,
                ins=[actloc[:].opt()], outs=[actfull[:].opt()])

            actA = mainp.tile([128, S], BF16, tag="A")
            actB = mainp.tile([65, S], BF16, tag="C")
            nc.gpsimd.memset(actB[64:65, :], 1.0)
            af = actfull[:]
            for q in range(4):
                sl = bass.ts(q, S // 4)
                nc.sync.dma_start(actA[:, sl], af[0:128, sl])
                nc.scalar.dma_start(actB[0:64, sl], af[128:192, sl])

            do3 = d_out.rearrange("(t p) c -> t p c", p=128)
            with tc.tile_pool(name="ops", bufs=1, space="PSUM") as ops, \
                 tc.tile_pool(name="osb", bufs=1) as osb:
                for tg in range(32):
                    po = ops.tile([128, 4 * CL], F32, tag="out", bufs=2)
                    for i in range(4):
                        t = tg * 4 + i
                        tsl = bass.ts(t, 128)
                        nc.tensor.matmul(po[:, bass.ts(i, CL)], actA[:, tsl],
                                         woA[:], start=True, stop=False)
                        nc.tensor.matmul(po[:, bass.ts(i, CL)], actB[:, tsl],
                                         woB[:], start=False, stop=True)
                    ob = osb.tile([128, 4 * CL], BF16, tag="ob", bufs=3)
                    if tg % 2 == 0:
                        nc.vector.tensor_copy(ob[:], po[:])
                    else:
                        nc.scalar.activation(ob[:], po[:], AF.Identity)
                    ob3 = ob.rearrange("p (i c) -> p i c", i=4)
                    for i in range(4):
                        eng = nc.sync if (tg + i) % 2 == 0 else nc.scalar
                        eng.dma_start(do3[tg * 4 + i], ob3[:, i, :])
    return nc


def _prep_weights(mask_h, mask_w, Wq, bq, Wk, bk, Wv, bv, lepe_w, lepe_b, Wo, bo):
    """Per-core weight-derived inputs, concatenated to (8*rows, cols) globals."""
    Wk_s = Wk * SCALING
    bk_s = bk * SCALING
    bo_eff = bo + lepe_b @ Wo
    names = ["wq0", "wq1", "wk0", "wk1", "wv0", "wv1", "emw", "emh",
             "ldiag", "woA", "woB"]
    per = {n: [] for n in names}
    cc = np.arange(CL)
    for core in range(8):
        g = core // 4
        ch = slice(g * CL, (g + 1) * CL)
        hd = slice(g * NHL, (g + 1) * NHL)
        wq_ext = np.concatenate([Wq[:, ch], bq[None, ch]], 0)    # (193, 96)
        wk_ext = np.concatenate([Wk_s[:, ch], bk_s[None, ch]], 0)
        wv_ext = np.concatenate([Wv[:, ch], bv[None, ch]], 0)
        # mask[n] is (row, col) with softmax over col; scoresT tiles are
        # (col ; row), so take exp(mask).T per head -> (u ; n, w)
        emw = np.exp(mask_w[0, hd]).transpose(2, 0, 1).reshape(128, NHL * W)
        emh = np.exp(mask_h[0, hd]).transpose(2, 0, 1).reshape(128, NHL * H)
        kk = lepe_w[:, :, 0, ch].reshape(25, CL)                 # (25, 96)
        ld = np.zeros((CL, 25 * 32), np.float32)
        for t in range(25):
            ld[cc, t * 32 + (cc % 32)] = kk[t]
        per["wq0"].append(wq_ext[:128]); per["wq1"].append(wq_ext[128:])
        per["wk0"].append(wk_ext[:128]); per["wk1"].append(wk_ext[128:])
        per["wv0"].append(wv_ext[:128]); per["wv1"].append(wv_ext[128:])
        per["emw"].append(emw); per["emh"].append(emh)
        per["ldiag"].append(ld)
        per["woA"].append(Wo[0:128, ch])
        per["woB"].append(np.concatenate([Wo[128:192, ch], bo_eff[None, ch]], 0))
    return {n: _bf16(np.concatenate(per[n], axis=0)) for n in names}


def _prep_x(x):
    xb = x.astype(ml_dtypes.bfloat16)                    # (4,128,128,192)
    g = np.empty((8, S // 2, C), ml_dtypes.bfloat16)
    g[0:4] = xb[:, :H // 2].reshape(4, S // 2, C)
    g[4:8] = xb[:, H // 2:].reshape(4, S // 2, C)
    return g.reshape(8 * (S // 2), C)


def _crc(*arrays):
    h = 0
    for a in arrays:
        h = zlib.crc32(np.ascontiguousarray(a).view(np.uint8).reshape(-1), h)
    return h


def _get_state():
    st = _CACHE.get("state")
    if st is not None:
        return st
    nc = bacc.Bacc("TRN2", target_bir_lowering=False, debug=False,
                   num_devices=8)
    build(nc)
    nc.compile()
    b2j.install_neuronx_cc_hook()

    partition_name = (nc.partition_id_tensor.name
                      if nc.partition_id_tensor else None)
    in_names, out_names, out_avals = [], [], []
    for alloc in nc.m.functions[0].allocations:
        if not isinstance(alloc, mybir.MemoryLocationSet):
            continue
        name = alloc.memorylocations[0].name
        if alloc.kind == "ExternalInput":
            if name != partition_name:
                in_names.append(name)
        elif alloc.kind == "ExternalOutput":
            out_names.append(name)
            out_avals.append(jax.core.ShapedArray(
                tuple(alloc.tensor_shape), mybir.dt.np(alloc.dtype)))
    all_in_names = list(in_names) + list(out_names)
    if partition_name is not None:
        all_in_names.append(partition_name)

    def _body(*args):
        operands = list(args)
        if partition_name is not None:
            operands.append(b2j.partition_id_tensor())
        outs = b2j._bass_exec_p.bind(
            *operands, out_avals=tuple(out_avals),
            in_names=tuple(all_in_names), out_names=tuple(out_names),
            lowering_input_output_aliases=(),
            sim_require_finite=True, sim_require_nnan=True, nc=nc)
        return tuple(outs)

    devices = jax.devices()[:8]
    mesh = Mesh(np.asarray(devices), ("core",))
    n_in_all = len(in_names) + len(out_names)
    sharded = jax.jit(
        shard_map(_body, mesh=mesh,
                  in_specs=(PartitionSpec("core"),) * n_in_all,
                  out_specs=(PartitionSpec("core"),) * len(out_names),
                  check_rep=False),
        keep_unused=True)
    sh = NamedSharding(mesh, PartitionSpec("core"))
    # output-slot placeholder operands (content never observed: the kernel
    # writes every element of each output)
    dev_zeros = [jax.device_put(
        np.zeros((8 * a.shape[0], *a.shape[1:]), a.dtype), sh)
        for a in out_avals]
    st = {"nc": nc, "sharded": sharded, "sh": sh, "in_names": in_names,
          "dev_zeros": dev_zeros, "wkey": None, "xkey": None,
          "dev": {}}
    _CACHE["state"] = st
    return st


def kernel(x, mask_h, mask_w, Wq, bq, Wk, bk, Wv, bv, lepe_w, lepe_b, Wo, bo):
    x = np.asarray(x, np.float32)
    wlist = [np.asarray(a, np.float32) for a in
             (mask_h, mask_w, Wq, bq, Wk, bk, Wv, bv, lepe_w, lepe_b, Wo, bo)]
    st = _get_state()

    wkey = _crc(*wlist)
    if st["wkey"] != wkey:
        gw = _prep_weights(*wlist)
        for n, a in gw.items():
            st["dev"][n] = jax.device_put(a, st["sh"])
        st["wkey"] = wkey
    xkey = _crc(x)
    if st["xkey"] != xkey:
        st["dev"]["xh"] = jax.device_put(_prep_x(x), st["sh"])
        st["xkey"] = xkey

    args = [st["dev"][n] for n in st["in_names"]] + st["dev_zeros"]
    out_raw = np.asarray(st["sharded"](*args)[0])      # (8*16384, 96) bf16
    og = out_raw.reshape(2, 4, S, CL)
    res = np.empty((4, S, C), np.float32)
    res[:, :, 0:CL] = og[0]
    res[:, :, CL:C] = og[1]
    return res.reshape(B, H, W, C)
